# revision 11
# baseline (speedup 1.0000x reference)
"""Causal self-attention Trainium2 kernel (v2).

Full inputs -> full outputs. Data-parallel over batch across 8 NeuronCores
(16 batches per core), no collectives.

Differences vs v1 (which ran at ~887us, PE 33% busy, latency-bound on a
per-(batch,head) DRAM round-trip normalization chain):
  - X^T is built on the HOST and DMA'd feature-on-partition directly:
    no PE transposes, no transpose evictions.
  - Normalization is fully on-chip: Z row sits in PSUM row 64 (ones
    column in the PV lhsT), DVE reciprocal reads it from PSUM, GpSimd
    partition_broadcast spreads 1/Z across 64 partitions, one DVE
    multiply writes the normalized O^T. Zero DMAs in the chain.
  - Projection is head-pair packed (K=128): even heads' O^T lands in
    OTP[0:64], odd heads' in a staging tile that one SBUF->SBUF DMA per
    group lifts to OTP[64:128]. Halves the projection matmul count.
  - Output bias is injected with a K=1 ones x beff matmul so the Y
    eviction is a plain scalar copy (keeps DVE load balanced).

Layout per core (everything f32, matmuls consume f32r at N>=256 so the
PE streams 1 column/cycle):
  - Q^T/K^T [feature, tok] via weight-stationary matmuls over X^T;
    1/sqrt(hd) and biases folded into the PSUM->SBUF eviction.
  - V [tok, feature] with an interleaved ones column per head.
  - Scores transposed: S^T[k, q] = K^T.T @ Q^T with the causal -1e30
    mask pre-added into PSUM via an identity matmul, then exp (ACT).
"""

import numpy as np

import concourse.bass as bass
import concourse.bacc as bacc
import concourse.mybir as mybir
import concourse.tile as tile

N_CORES = 8
B, T, C = 128, 256, 384
H, HD = 6, 64
NB = B // N_CORES          # batches per core
TOK = NB * T               # tokens per core
G = 2                      # batches per group
NG = NB // G               # groups per core
GT = G * T                 # tokens per group (512)
NTT = GT // 128            # 128-token tiles per group (4)
F32 = mybir.dt.float32
F32R = mybir.dt.float32r
AF = mybir.ActivationFunctionType
NEGBIG = -1.0e30


def _body(tc, xT_d, wat_d, wpt_d, bq_d, bk_d, beff_d, mask_d,
          identr_d, onescol_d, y_d):
    nc = tc.nc
    from contextlib import ExitStack

    ctx = ExitStack()
    with ctx:
        const = ctx.enter_context(tc.tile_pool(name="const", bufs=1))
        xin = ctx.enter_context(tc.tile_pool(name="xin", bufs=2))
        qkt = ctx.enter_context(tc.tile_pool(name="qkt", bufs=2))
        v65 = ctx.enter_context(tc.tile_pool(name="v65", bufs=2))
        pp = ctx.enter_context(tc.tile_pool(name="pp", bufs=4))
        rp = ctx.enter_context(tc.tile_pool(name="rp", bufs=4))
        rbcp = ctx.enter_context(tc.tile_pool(name="rbcp", bufs=4))
        otp_pool = ctx.enter_context(tc.tile_pool(name="otp", bufs=2))
        oto_pool = ctx.enter_context(tc.tile_pool(name="oto", bufs=2))
        yb = ctx.enter_context(tc.tile_pool(name="yb", bufs=3))
        mm_ps = ctx.enter_context(tc.tile_pool(name="mm_ps", bufs=3, space="PSUM"))
        s_ps = ctx.enter_context(tc.tile_pool(name="s_ps", bufs=2, space="PSUM"))
        o_ps = ctx.enter_context(tc.tile_pool(name="o_ps", bufs=3, space="PSUM"))

        dma = nc.sync.dma_start

        wat_sb = const.tile([128, 3, 3 * C], F32R, name="wat_sb")
        wpt_sb = const.tile([128, 3, C], F32R, name="wpt_sb")
        bq_sb = const.tile([128, 3], F32, name="bq_sb")
        bk_sb = const.tile([128, 3], F32, name="bk_sb")
        beff_sb = const.tile([128, C], F32, name="beff_sb")
        mask_sb = const.tile([128, 2 * T], F32R, name="mask_sb")
        identr_sb = const.tile([128, 128], F32R, name="identr_sb")
        onescol_sb = const.tile([128, 1], F32R, name="onescol_sb")

        dma(wat_sb[:], wat_d.ap().rearrange("(ct p) f -> p ct f", p=128))
        dma(wpt_sb[:], wpt_d.ap())
        dma(bq_sb[:], bq_d.ap())
        dma(bk_sb[:], bk_d.ap())
        dma(beff_sb[:], beff_d.ap())
        dma(mask_sb[:], mask_d.ap())
        dma(identr_sb[:], identr_d.ap())
        dma(onescol_sb[:], onescol_d.ap())

        xv = xT_d.ap().rearrange("p ct (g t) -> g p ct t", g=NG)
        yv = y_d.ap().rearrange("(g tt p) c -> g tt p c", tt=NTT, p=128)

        def load_x(g):
            t = xin.tile([128, 3, GT], F32R, name=f"xt_{g}", tag="xt")
            dma(t[:], xv[g])
            return t

        def emit_proj(OTP_prev, gp):
            # ---- projection [tok, C], bias via K=1 ones x beff matmul
            for tt in range(NTT):
                ps_y = mm_ps.tile([128, 384], F32, name=f"psy_{gp}_{tt}", tag="mm")
                for hp in range(3):
                    nc.tensor.matmul(
                        ps_y[:],
                        OTP_prev[:, hp, 128 * tt:128 * (tt + 1)],
                        wpt_sb[:, hp, :],
                        start=(hp == 0),
                        stop=(hp == 2),
                    )
                Y_sb = yb.tile([128, C], F32, name=f"Y_{gp}_{tt}", tag="Y")
                nc.vector.tensor_add(Y_sb[:], ps_y[:], beff_sb[:])
                dma(yv[gp, tt], Y_sb[:])

        xt_next = load_x(0)
        pending = None
        for g in range(NG):
            xt = xt_next
            if g + 1 < NG:
                xt_next = load_x(g + 1)

            # ---- Q^T / K^T  [feature, tok]
            qk = qkt.tile([128, 6, GT], F32R, name=f"qk_{g}", tag="qk")
            for ft in range(6):
                ps = mm_ps.tile([128, GT], F32, name=f"psqk_{g}_{ft}", tag="mm")
                for ct in range(3):
                    nc.tensor.matmul(
                        ps[:],
                        wat_sb[:, ct, 128 * ft:128 * (ft + 1)],
                        xt[:, ct, :],
                        start=(ct == 0),
                        stop=(ct == 2),
                    )
                if ft < 3:
                    nc.scalar.activation(qk[:, ft, :], ps[:], AF.Identity,
                                         bias=bq_sb[:, ft:ft + 1], scale=0.125)
                else:
                    nc.scalar.activation(qk[:, ft, :], ps[:], AF.Identity,
                                         bias=bk_sb[:, ft - 3:ft - 2], scale=1.0)

            # ---- V [tok, feature]
            V64_sb = v65.tile([128, NTT, H * 64], F32R, name=f"V64_{g}", tag="V64")
            for tt in range(NTT):
                psv = mm_ps.tile([128, 384], F32, name=f"psv_{g}_{tt}", tag="mm")
                for ct in range(3):
                    nc.tensor.matmul(
                        psv[:],
                        xt[:, ct, 128 * tt:128 * (tt + 1)],
                        wat_sb[:, ct, 2 * C:3 * C],
                        start=(ct == 0),
                        stop=(ct == 2),
                    )
                nc.vector.tensor_copy(V64_sb[:, tt, :], psv[:])

            if pending is not None:
                emit_proj(*pending)

            # ---- attention per (batch, head-pair)
            OTP_sb = otp_pool.tile([128, 3, GT], F32R, name=f"OTP_{g}", tag="OTP")
            OTO_sb = oto_pool.tile([64, 3, GT], F32R, name=f"OTO_{g}", tag="OTO")
            for bl in range(G):
                q0 = 256 * bl
                for hp in range(3):
                    ps_pair = []
                    for par in range(2):
                        ps_s = s_ps.tile([128, 512], F32,
                                         name=f"pss_{g}_{bl}_{hp}_{par}", tag="s")
                        ps_pair.append(ps_s)
                        nc.tensor.matmul(ps_s[:], identr_sb[:], mask_sb[:],
                                         start=True, stop=False)
                    for kt in range(2):
                        for par in range(2):
                            row0 = 64 * par
                            KT = qk[row0:row0 + 64, 3 + hp, :]
                            QT = qk[row0:row0 + 64, hp, q0:q0 + 256]
                            nc.tensor.matmul(
                                ps_pair[par][:, 256 * kt:256 * (kt + 1)],
                                KT[:, q0 + 128 * kt:q0 + 128 * (kt + 1)],
                                QT,
                                start=False,
                                stop=(kt == 1),
                            )
                    for par in range(2):
                        h = 2 * hp + par
                        P_sb = pp.tile([128, 512], F32R,
                                       name=f"P_{g}_{bl}_{h}", tag="P")
                        nc.scalar.activation(P_sb[:], ps_pair[par][:], AF.Exp)
                        ps_o = o_ps.tile([128, 512], F32,
                                         name=f"pso_{g}_{bl}_{h}", tag="o")
                        nc.tensor.matmul(ps_o[0:64, 0:256],
                                         V64_sb[:, 2 * bl, 64 * h:64 * h + 64],
                                         P_sb[:, 0:256], start=True, stop=False)
                        nc.tensor.matmul(ps_o[0:64, 0:256],
                                         V64_sb[:, 2 * bl + 1, 64 * h:64 * h + 64],
                                         P_sb[:, 256:512], start=False, stop=True)
                        # Z = column sums of P, landed on partition 0 in the
                        # spare half of the same PSUM bank
                        nc.tensor.matmul(ps_o[0:1, 256:512], onescol_sb[:],
                                         P_sb[:, 0:256], start=True, stop=False,
                                         skip_group_check=True)
                        nc.tensor.matmul(ps_o[0:1, 256:512], onescol_sb[:],
                                         P_sb[:, 256:512], start=False, stop=True,
                                         skip_group_check=True)
                        # on-chip normalization, everything at base partition 0
                        r_sb = rp.tile([1, 256], F32, name=f"r_{g}_{bl}_{h}",
                                       tag="r")
                        nc.vector.reciprocal_approx_fast(r_sb[:],
                                                         ps_o[0:1, 256:512])
                        rbc_sb = rbcp.tile([64, 256], F32,
                                           name=f"rbc_{g}_{bl}_{h}", tag="rbc")
                        nc.gpsimd.partition_broadcast(rbc_sb[:], r_sb[:])
                        if par == 0:
                            dst = OTP_sb[0:64, hp, q0:q0 + 256]
                        else:
                            dst = OTO_sb[0:64, hp, q0:q0 + 256]
                        nc.vector.tensor_mul(dst, ps_o[0:64, 0:256], rbc_sb[:])

            # lift odd heads' O^T onto partitions 64:128 for K=128 projection
            dma(OTP_sb[64:128, :, :], OTO_sb[:])
            pending = (OTP_sb, g)

        emit_proj(*pending)


_CACHE = {}


def _build_nc():
    if "nc" in _CACHE:
        return _CACHE["nc"]
    nc = bacc.Bacc("TRN2", target_bir_lowering=False, debug=False,
                   num_devices=N_CORES)
    xT_d = nc.dram_tensor("xT", [128, 3, TOK], F32R, kind="ExternalInput")
    wat_d = nc.dram_tensor("w_attnT", [C, 3 * C], F32R, kind="ExternalInput")
    wpt_d = nc.dram_tensor("w_projT", [128, 3, C], F32R, kind="ExternalInput")
    bq_d = nc.dram_tensor("bq", [128, 3], F32, kind="ExternalInput")
    bk_d = nc.dram_tensor("bk", [128, 3], F32, kind="ExternalInput")
    beff_d = nc.dram_tensor("beff", [128, C], F32, kind="ExternalInput")
    mask_d = nc.dram_tensor("maskS", [128, 2 * T], F32R, kind="ExternalInput")
    identr_d = nc.dram_tensor("identr", [128, 128], F32R, kind="ExternalInput")
    onescol_d = nc.dram_tensor("onescol", [128, 1], F32R, kind="ExternalInput")
    y_d = nc.dram_tensor("y", [TOK, C], F32, kind="ExternalOutput")

    with tile.TileContext(nc) as tc:
        _body(tc, xT_d, wat_d, wpt_d, bq_d, bk_d, beff_d, mask_d,
              identr_d, onescol_d, y_d)
    nc.compile()
    _CACHE["nc"] = nc
    return nc


def _host_inputs(x, w_attn, b_attn, w_proj, b_proj):
    """Build the per-core input maps (host-side prep of weights/constants)."""
    w_attnT = np.ascontiguousarray(w_attn.T)                       # [C, 3C]
    # w_projT regrouped per head-pair: wpt[p, hp, of] = w_proj[of, 128*hp+p]
    wpt = np.ascontiguousarray(w_proj.T.reshape(3, 128, C).transpose(1, 0, 2))
    bq = np.ascontiguousarray((0.125 * b_attn[:C]).reshape(3, 128).T)
    bk = np.ascontiguousarray(b_attn[C:2 * C].reshape(3, 128).T)
    b_eff = b_proj + w_proj @ b_attn[2 * C:]
    beff = np.ascontiguousarray(np.broadcast_to(b_eff, (128, C))).astype(np.float32)

    # mask for S^T bank [128, 512]: cols j<256: (k=p, q=j); cols j>=256:
    # (k=128+p, q=j-256)
    p = np.arange(128)[:, None]
    j = np.arange(512)[None, :]
    valid = np.where(j < 256, p <= j, p <= j - 384)
    mask = np.where(valid, 0.0, NEGBIG).astype(np.float32)
    ident = np.eye(128, dtype=np.float32)

    common = {
        "w_attnT": w_attnT.astype(np.float32),
        "w_projT": wpt.astype(np.float32),
        "bq": bq.astype(np.float32),
        "bk": bk.astype(np.float32),
        "beff": beff,
        "maskS": mask,
        "identr": ident,
        "onescol": np.ones((128, 1), dtype=np.float32),
    }
    xs = x.reshape(N_CORES, TOK, C)
    in_maps = []
    for c in range(N_CORES):
        # xT[p, ct, t] = x_core[t, 128*ct + p]
        xT = np.ascontiguousarray(
            xs[c].T.reshape(3, 128, TOK).transpose(1, 0, 2)
        ).astype(np.float32)
        m = dict(common)
        m["xT"] = xT
        in_maps.append(m)
    return in_maps


def kernel(x, w_attn, b_attn, w_proj, b_proj):
    from concourse.bass_utils import run_bass_kernel_spmd

    x = np.asarray(x, dtype=np.float32)
    w_attn = np.asarray(w_attn, dtype=np.float32)
    b_attn = np.asarray(b_attn, dtype=np.float32)
    w_proj = np.asarray(w_proj, dtype=np.float32)
    b_proj = np.asarray(b_proj, dtype=np.float32)

    nc = _build_nc()
    in_maps = _host_inputs(x, w_attn, b_attn, w_proj, b_proj)
    res = run_bass_kernel_spmd(nc, in_maps, core_ids=list(range(N_CORES)))
    y = np.stack([res.results[c]["y"] for c in range(N_CORES)])
    return y.reshape(B, T, C)


# revision 12
# speedup vs baseline: 1.0314x; 1.0314x over previous
"""Causal self-attention Trainium2 kernel (v2).

Full inputs -> full outputs. Data-parallel over batch across 8 NeuronCores
(16 batches per core), no collectives.

Differences vs v1 (which ran at ~887us, PE 33% busy, latency-bound on a
per-(batch,head) DRAM round-trip normalization chain):
  - X^T is built on the HOST and DMA'd feature-on-partition directly:
    no PE transposes, no transpose evictions.
  - Normalization is fully on-chip: Z row sits in PSUM row 64 (ones
    column in the PV lhsT), DVE reciprocal reads it from PSUM, GpSimd
    partition_broadcast spreads 1/Z across 64 partitions, one DVE
    multiply writes the normalized O^T. Zero DMAs in the chain.
  - Projection is head-pair packed (K=128): even heads' O^T lands in
    OTP[0:64], odd heads' in a staging tile that one SBUF->SBUF DMA per
    group lifts to OTP[64:128]. Halves the projection matmul count.
  - Output bias is injected with a K=1 ones x beff matmul so the Y
    eviction is a plain scalar copy (keeps DVE load balanced).

Layout per core (everything f32, matmuls consume f32r at N>=256 so the
PE streams 1 column/cycle):
  - Q^T/K^T [feature, tok] via weight-stationary matmuls over X^T;
    1/sqrt(hd) and biases folded into the PSUM->SBUF eviction.
  - V [tok, feature] with an interleaved ones column per head.
  - Scores transposed: S^T[k, q] = K^T.T @ Q^T with the causal -1e30
    mask pre-added into PSUM via an identity matmul, then exp (ACT).
"""

import numpy as np

import concourse.bass as bass
import concourse.bacc as bacc
import concourse.mybir as mybir
import concourse.tile as tile

N_CORES = 8
B, T, C = 128, 256, 384
H, HD = 6, 64
NB = B // N_CORES          # batches per core
TOK = NB * T               # tokens per core
G = 2                      # batches per group
NG = NB // G               # groups per core
GT = G * T                 # tokens per group (512)
NTT = GT // 128            # 128-token tiles per group (4)
F32 = mybir.dt.float32
F32R = mybir.dt.float32r
AF = mybir.ActivationFunctionType
NEGBIG = -1.0e30


def _body(tc, xT_d, wat_d, wpt_d, bq_d, bk_d, beff_d, mask_d,
          identr_d, onescol_d, y_d):
    nc = tc.nc
    from contextlib import ExitStack

    ctx = ExitStack()
    with ctx:
        const = ctx.enter_context(tc.tile_pool(name="const", bufs=1))
        xin = ctx.enter_context(tc.tile_pool(name="xin", bufs=3))
        qkt = ctx.enter_context(tc.tile_pool(name="qkt", bufs=2))
        v65 = ctx.enter_context(tc.tile_pool(name="v65", bufs=2))
        pp = ctx.enter_context(tc.tile_pool(name="pp", bufs=6))
        rp = ctx.enter_context(tc.tile_pool(name="rp", bufs=6))
        rbcp = ctx.enter_context(tc.tile_pool(name="rbcp", bufs=6))
        otp_pool = ctx.enter_context(tc.tile_pool(name="otp", bufs=2))
        oto_pool = ctx.enter_context(tc.tile_pool(name="oto", bufs=2))
        yb = ctx.enter_context(tc.tile_pool(name="yb", bufs=3))
        mm_ps = ctx.enter_context(tc.tile_pool(name="mm_ps", bufs=2, space="PSUM"))
        s_ps = ctx.enter_context(tc.tile_pool(name="s_ps", bufs=2, space="PSUM"))
        o_ps = ctx.enter_context(tc.tile_pool(name="o_ps", bufs=4, space="PSUM"))

        dma = nc.sync.dma_start

        wat_sb = const.tile([128, 3, 3 * C], F32R, name="wat_sb")
        wpt_sb = const.tile([128, 3, C], F32R, name="wpt_sb")
        bq_sb = const.tile([128, 3], F32, name="bq_sb")
        bk_sb = const.tile([128, 3], F32, name="bk_sb")
        beff_sb = const.tile([128, C], F32, name="beff_sb")
        mask_sb = const.tile([128, 2 * T], F32R, name="mask_sb")
        identr_sb = const.tile([128, 128], F32R, name="identr_sb")
        onescol_sb = const.tile([128, 1], F32R, name="onescol_sb")

        dma(wat_sb[:], wat_d.ap().rearrange("(ct p) f -> p ct f", p=128))
        dma(wpt_sb[:], wpt_d.ap())
        dma(bq_sb[:], bq_d.ap())
        dma(bk_sb[:], bk_d.ap())
        dma(beff_sb[:], beff_d.ap())
        dma(mask_sb[:], mask_d.ap())
        dma(identr_sb[:], identr_d.ap())
        dma(onescol_sb[:], onescol_d.ap())

        xv = xT_d.ap().rearrange("p ct (g t) -> g p ct t", g=NG)
        yv = y_d.ap().rearrange("(g tt p) c -> g tt p c", tt=NTT, p=128)

        def load_x(g):
            t = xin.tile([128, 3, GT], F32R, name=f"xt_{g}", tag="xt")
            dma(t[:], xv[g])
            return t

        def emit_proj(OTP_prev, gp):
            # ---- projection [tok, C], bias via K=1 ones x beff matmul
            for tt in range(NTT):
                ps_y = mm_ps.tile([128, 384], F32, name=f"psy_{gp}_{tt}", tag="mm")
                for hp in range(3):
                    nc.tensor.matmul(
                        ps_y[:],
                        OTP_prev[:, hp, 128 * tt:128 * (tt + 1)],
                        wpt_sb[:, hp, :],
                        start=(hp == 0),
                        stop=(hp == 2),
                    )
                Y_sb = yb.tile([128, C], F32, name=f"Y_{gp}_{tt}", tag="Y")
                nc.vector.tensor_add(Y_sb[:], ps_y[:], beff_sb[:])
                dma(yv[gp, tt], Y_sb[:])

        xt_next = load_x(0)
        pending = None
        for g in range(NG):
            xt = xt_next
            if g + 1 < NG:
                xt_next = load_x(g + 1)

            # ---- Q^T / K^T  [feature, tok]
            qk = qkt.tile([128, 6, GT], F32R, name=f"qk_{g}", tag="qk")
            for ft in range(6):
                ps = mm_ps.tile([128, GT], F32, name=f"psqk_{g}_{ft}", tag="mm")
                for ct in range(3):
                    nc.tensor.matmul(
                        ps[:],
                        wat_sb[:, ct, 128 * ft:128 * (ft + 1)],
                        xt[:, ct, :],
                        start=(ct == 0),
                        stop=(ct == 2),
                    )
                if ft < 3:
                    nc.scalar.activation(qk[:, ft, :], ps[:], AF.Identity,
                                         bias=bq_sb[:, ft:ft + 1], scale=0.125)
                else:
                    nc.scalar.activation(qk[:, ft, :], ps[:], AF.Identity,
                                         bias=bk_sb[:, ft - 3:ft - 2], scale=1.0)

            # ---- V [tok, feature]
            V64_sb = v65.tile([128, NTT, H * 64], F32R, name=f"V64_{g}", tag="V64")
            for tt in range(NTT):
                psv = mm_ps.tile([128, 384], F32, name=f"psv_{g}_{tt}", tag="mm")
                for ct in range(3):
                    nc.tensor.matmul(
                        psv[:],
                        xt[:, ct, 128 * tt:128 * (tt + 1)],
                        wat_sb[:, ct, 2 * C:3 * C],
                        start=(ct == 0),
                        stop=(ct == 2),
                    )
                nc.vector.tensor_copy(V64_sb[:, tt, :], psv[:])

            if pending is not None:
                emit_proj(*pending)

            # ---- attention per (batch, head-pair)
            OTP_sb = otp_pool.tile([128, 3, GT], F32R, name=f"OTP_{g}", tag="OTP")
            OTO_sb = oto_pool.tile([64, 3, GT], F32R, name=f"OTO_{g}", tag="OTO")
            for bl in range(G):
                q0 = 256 * bl
                for hp in range(3):
                    ps_pair = []
                    for par in range(2):
                        ps_s = s_ps.tile([128, 512], F32,
                                         name=f"pss_{g}_{bl}_{hp}_{par}", tag="s")
                        ps_pair.append(ps_s)
                        nc.tensor.matmul(ps_s[:], identr_sb[:], mask_sb[:],
                                         start=True, stop=False)
                    for kt in range(2):
                        for par in range(2):
                            row0 = 64 * par
                            KT = qk[row0:row0 + 64, 3 + hp, :]
                            QT = qk[row0:row0 + 64, hp, q0:q0 + 256]
                            nc.tensor.matmul(
                                ps_pair[par][:, 256 * kt:256 * (kt + 1)],
                                KT[:, q0 + 128 * kt:q0 + 128 * (kt + 1)],
                                QT,
                                start=False,
                                stop=(kt == 1),
                            )
                    for par in range(2):
                        h = 2 * hp + par
                        P_sb = pp.tile([128, 512], F32R,
                                       name=f"P_{g}_{bl}_{h}", tag="P")
                        nc.scalar.activation(P_sb[:], ps_pair[par][:], AF.Exp)
                        ps_o = o_ps.tile([128, 512], F32,
                                         name=f"pso_{g}_{bl}_{h}", tag="o")
                        nc.tensor.matmul(ps_o[0:64, 0:256],
                                         V64_sb[:, 2 * bl, 64 * h:64 * h + 64],
                                         P_sb[:, 0:256], start=True, stop=False)
                        nc.tensor.matmul(ps_o[0:64, 0:256],
                                         V64_sb[:, 2 * bl + 1, 64 * h:64 * h + 64],
                                         P_sb[:, 256:512], start=False, stop=True)
                        # Z = column sums of P, landed on partition 0 in the
                        # spare half of the same PSUM bank
                        nc.tensor.matmul(ps_o[0:1, 256:512], onescol_sb[:],
                                         P_sb[:, 0:256], start=True, stop=False,
                                         skip_group_check=True)
                        nc.tensor.matmul(ps_o[0:1, 256:512], onescol_sb[:],
                                         P_sb[:, 256:512], start=False, stop=True,
                                         skip_group_check=True)
                        # on-chip normalization, everything at base partition 0
                        r_sb = rp.tile([1, 256], F32, name=f"r_{g}_{bl}_{h}",
                                       tag="r")
                        nc.vector.reciprocal_approx_fast(r_sb[:],
                                                         ps_o[0:1, 256:512])
                        rbc_sb = rbcp.tile([64, 256], F32,
                                           name=f"rbc_{g}_{bl}_{h}", tag="rbc")
                        nc.gpsimd.partition_broadcast(rbc_sb[:], r_sb[:])
                        if par == 0:
                            dst = OTP_sb[0:64, hp, q0:q0 + 256]
                        else:
                            dst = OTO_sb[0:64, hp, q0:q0 + 256]
                        nc.vector.tensor_mul(dst, ps_o[0:64, 0:256], rbc_sb[:])

            # lift odd heads' O^T onto partitions 64:128 for K=128 projection
            dma(OTP_sb[64:128, :, :], OTO_sb[:])
            pending = (OTP_sb, g)

        emit_proj(*pending)


_CACHE = {}


def _build_nc():
    if "nc" in _CACHE:
        return _CACHE["nc"]
    nc = bacc.Bacc("TRN2", target_bir_lowering=False, debug=False,
                   num_devices=N_CORES)
    xT_d = nc.dram_tensor("xT", [128, 3, TOK], F32R, kind="ExternalInput")
    wat_d = nc.dram_tensor("w_attnT", [C, 3 * C], F32R, kind="ExternalInput")
    wpt_d = nc.dram_tensor("w_projT", [128, 3, C], F32R, kind="ExternalInput")
    bq_d = nc.dram_tensor("bq", [128, 3], F32, kind="ExternalInput")
    bk_d = nc.dram_tensor("bk", [128, 3], F32, kind="ExternalInput")
    beff_d = nc.dram_tensor("beff", [128, C], F32, kind="ExternalInput")
    mask_d = nc.dram_tensor("maskS", [128, 2 * T], F32R, kind="ExternalInput")
    identr_d = nc.dram_tensor("identr", [128, 128], F32R, kind="ExternalInput")
    onescol_d = nc.dram_tensor("onescol", [128, 1], F32R, kind="ExternalInput")
    y_d = nc.dram_tensor("y", [TOK, C], F32, kind="ExternalOutput")

    with tile.TileContext(nc) as tc:
        _body(tc, xT_d, wat_d, wpt_d, bq_d, bk_d, beff_d, mask_d,
              identr_d, onescol_d, y_d)
    nc.compile()
    _CACHE["nc"] = nc
    return nc


def _host_inputs(x, w_attn, b_attn, w_proj, b_proj):
    """Build the per-core input maps (host-side prep of weights/constants)."""
    w_attnT = np.ascontiguousarray(w_attn.T)                       # [C, 3C]
    # w_projT regrouped per head-pair: wpt[p, hp, of] = w_proj[of, 128*hp+p]
    wpt = np.ascontiguousarray(w_proj.T.reshape(3, 128, C).transpose(1, 0, 2))
    bq = np.ascontiguousarray((0.125 * b_attn[:C]).reshape(3, 128).T)
    bk = np.ascontiguousarray(b_attn[C:2 * C].reshape(3, 128).T)
    b_eff = b_proj + w_proj @ b_attn[2 * C:]
    beff = np.ascontiguousarray(np.broadcast_to(b_eff, (128, C))).astype(np.float32)

    # mask for S^T bank [128, 512]: cols j<256: (k=p, q=j); cols j>=256:
    # (k=128+p, q=j-256)
    p = np.arange(128)[:, None]
    j = np.arange(512)[None, :]
    valid = np.where(j < 256, p <= j, p <= j - 384)
    mask = np.where(valid, 0.0, NEGBIG).astype(np.float32)
    ident = np.eye(128, dtype=np.float32)

    common = {
        "w_attnT": w_attnT.astype(np.float32),
        "w_projT": wpt.astype(np.float32),
        "bq": bq.astype(np.float32),
        "bk": bk.astype(np.float32),
        "beff": beff,
        "maskS": mask,
        "identr": ident,
        "onescol": np.ones((128, 1), dtype=np.float32),
    }
    xs = x.reshape(N_CORES, TOK, C)
    in_maps = []
    for c in range(N_CORES):
        # xT[p, ct, t] = x_core[t, 128*ct + p]
        xT = np.ascontiguousarray(
            xs[c].T.reshape(3, 128, TOK).transpose(1, 0, 2)
        ).astype(np.float32)
        m = dict(common)
        m["xT"] = xT
        in_maps.append(m)
    return in_maps


def kernel(x, w_attn, b_attn, w_proj, b_proj):
    from concourse.bass_utils import run_bass_kernel_spmd

    x = np.asarray(x, dtype=np.float32)
    w_attn = np.asarray(w_attn, dtype=np.float32)
    b_attn = np.asarray(b_attn, dtype=np.float32)
    w_proj = np.asarray(w_proj, dtype=np.float32)
    b_proj = np.asarray(b_proj, dtype=np.float32)

    nc = _build_nc()
    in_maps = _host_inputs(x, w_attn, b_attn, w_proj, b_proj)
    res = run_bass_kernel_spmd(nc, in_maps, core_ids=list(range(N_CORES)))
    y = np.stack([res.results[c]["y"] for c in range(N_CORES)])
    return y.reshape(B, T, C)
